# revision 38
# baseline (speedup 1.0000x reference)
"""Trainium2 Bass kernel for nn_FRAP_47047071761018.

FRAP forward pass (demand MLP -> coupled-demand MLP -> D-tensor -> two
1x1-conv stacks -> (1,7) conv). Batch=1, tiny dims => latency problem.
Strategy: replicate the whole computation on all 8 cores (no useful
tensor sharding), read the result from core 0.

Device dataflow (float32r, features-on-partitions orientation):
  - every Linear is out = lhsT.T @ rhs on the PE with K on partitions;
    operands are float32r (single-pass PE fp32, ~2x the fp32 rate)
  - relu(x+b) fused into one DVE tensor_scalar (op0=add per-partition
    bias, op1=max 0); the second relu of 128-row pairs runs on ACT
  - host folds (weight preprocessing): demand-L3 + pair-gather +
    coupled-L1 -> FaT/FbT = (c_w1[:, :4] @ [d_w3|d_b3])^T, and
    Conv_D-L3 + Conv_C-L1 -> WeffT = (cC_w1[:, :2] @ dD_w3)^T with
    beff = cC_w1[:, :2] @ dD_b3 + cC_b1
  - D tensor as a (36, 56) tile: top channels at rows 0:4, bottom at
    rows 32:36 (legal DVE write bases), zero rows between, matched by
    zero-padded WD1T36 weights -> Conv_D L1 is 2 matmuls
  - final (1,7) conv = 7 accumulating matmuls with stride-7 rhs APs,
    bias via an extra ones-partition row
  - inputs arrive as 4 packed DRAM arrays over both HWDGE queues
    (SP + ACT), ordered by need time; the Tile exit tail is slimmed to
    just the completion drain (the Bass preamble re-clears semaphores
    at the start of every execution)
Pair rows are processed in a permuted order (PI) chosen so both lane
gathers are strided APs; the host unpermutes the 8 outputs at the end.
"""

import numpy as np

# ---------------------------------------------------------------------------
# static structure (from the FRAP definition; hardcoded, no file reads)
# ---------------------------------------------------------------------------
# reference PAIRS rows:
_PAIRS = [(0, 4), (1, 5), (2, 6), (3, 7), (0, 5), (1, 4), (2, 7), (3, 6)]
# our processing order: both column projections are strided-AP sequences
_PI_PAIRS = [(0, 4), (0, 5), (1, 4), (1, 5), (2, 6), (2, 7), (3, 6), (3, 7)]
_ORIGROW = [_PAIRS.index(p) for p in _PI_PAIRS]          # p' -> original row
_POS_OF_ORIG = np.argsort(_ORIGROW)                      # original row -> p'


def _build_bbot():
    """Bbot[q, p'*7+k] = 1 iff ORIGROW[q] == j(ORIGROW[p'], k), j=k+(k>=i)."""
    B = np.zeros((8, 56), np.float32)
    for pp in range(8):
        i = _ORIGROW[pp]
        for k in range(7):
            jj = k + (1 if k >= i else 0)
            B[_POS_OF_ORIG[jj], pp * 7 + k] = 1.0
    return B


def _host_pack(inp):
    """Marshal kernel inputs into the 7 packed DRAM arrays the device
    program expects (transposes / concats / 0-1 selection constants)."""
    f = lambda x: np.ascontiguousarray(np.asarray(x, np.float32))
    # --- a14 (14, 264): XT | W1dT ---
    XT = np.concatenate([
        np.broadcast_to(f(inp["i_position"])[0, 0], (1, 8)),
        np.broadcast_to(f(inp["j_position"])[0, 0], (1, 8)),
        f(inp["wtime_mu"]), f(inp["wtime_sigma"]),
        f(inp["wtime_max"]), f(inp["waiting"]),
        np.broadcast_to(f(inp["phase"]).T, (8, 8)),
    ], axis=0)
    a14 = np.concatenate([XT, f(inp["d_w1"]).T], axis=1)          # (14, 264)

    # --- a128 (128, 229): W2dT(64) WD2T(64) WC2T(64) Wc2T(32) b1(2) bD1(2) bc1(1)
    w2t = f(inp["d_w2"]).T
    wd2t = f(inp["dD_w2"]).T
    wc2t = f(inp["cC_w2"]).T
    beff = (f(inp["cC_w1"])[:, 0:2] @ f(inp["dD_b3"]) + f(inp["cC_b1"]))
    a128 = np.concatenate([
        w2t[:128], w2t[128:], wd2t[:128], wd2t[128:], wc2t[:128], wc2t[128:],
        f(inp["c_w2"]).T,
        f(inp["d_b1"]).reshape(2, 128).T, f(inp["dD_b1"]).reshape(2, 128).T,
        f(inp["c_b1"]).reshape(128, 1),
        beff.reshape(2, 128).T,
    ], axis=1)                                                    # (128, 231)
    wd1t = f(inp["dD_w1"]).T                                      # (8, 256)
    wd1t36 = np.zeros((128, 256), np.float32)
    wd1t36[0:4] = wd1t[0:4]
    wd1t36[32:36] = wd1t[4:8]
    a128 = np.concatenate([a128, wd1t36], axis=1)                 # (128, 487)

    # --- a33 (33, 528): [Wc3Te(4) b2 bD2 bc2 bc2C WcC3T(8)] | FaT(128) | FbT(128)
    #     | WeffT(256)  (host-folded weight products)
    a33 = np.zeros((33, 528), np.float32)
    a33[0:32, 0:4] = f(inp["c_w3"]).T[0:32]
    a33[32, 0:4] = f(inp["c_b3"])
    a33[0:32, 4] = f(inp["d_b2"])
    a33[0:32, 5] = f(inp["dD_b2"])
    a33[0:32, 6] = f(inp["c_b2"])
    a33[0:32, 7] = f(inp["cC_b2"])
    a33[0:32, 8:16] = f(inp["cC_w3"]).T
    w3aug = np.concatenate([f(inp["d_w3"]), f(inp["d_b3"])[:, None]], axis=1)  # (4,33)
    wc1 = f(inp["c_w1"])                                          # (128, 8)
    a33[0:33, 16:144] = (wc1[:, 0:4] @ w3aug).T
    a33[0:33, 144:272] = (wc1[:, 4:8] @ w3aug).T
    a33[0:32, 272:528] = (f(inp["cC_w1"])[:, 0:2] @ f(inp["dD_w3"])).T

    # --- a9 (9, 73): W4e 0:7 | Bbot 7:63 | bc3C 63 | b4 [0,64] | ones [0,65:73]
    a9 = np.zeros((9, 73), np.float32)
    a9[0:8, 0:7] = f(inp["cC_w4"])
    a9[0:8, 7:63] = _build_bbot()
    a9[0:8, 63] = f(inp["cC_b3"])
    a9[0, 64] = f(inp["cC_b4"])[0]
    a9[0, 65:73] = 1.0

    return {
        "a14": np.ascontiguousarray(a14), "a128": np.ascontiguousarray(a128),
        "a33": a33, "a9": a9,
    }


# ---------------------------------------------------------------------------
# bass program
# ---------------------------------------------------------------------------
_NC_CACHE = {}


def _build_nc():
    import concourse.bass as bass
    import concourse.bacc as bacc
    import concourse.mybir as mybir
    import concourse.tile as tile_mod
    from concourse.tile import TileContext
    from concourse.vector_clock import ScopedClock

    # Slim kernel tail: keep the completion drain (waits for the out-DMA
    # queue, guaranteeing the output is in DRAM before the NEFF retires)
    # but skip the two all-engine barriers + per-sem zeroing. Safe here
    # because the Bass preamble dma_reset/sem_clears the full kernel sem
    # range at the START of every execution, so re-exec state is clean.
    def _slim_drain_and_barrier(self, tick_clock, wait_clock):
        drain_inst = self.nc.sync.drain()
        wait_clock.add_sem_waits(
            drain_inst.ins, ScopedClock({None: tick_clock.global_clock})
        )
        popped = self.nc._tile_sem_poison_stack.pop()
        assert popped is self._sem_poison

    F32 = mybir.dt.float32
    ADD = mybir.AluOpType.add
    MAX = mybir.AluOpType.max

    # Bacc (not raw Bass): its compile() runs move_matmul_waits_to_ldweights
    # + generate_event_semaphores, which split multi-wait instructions into
    # the 1-wait form TRN2 walrus codegen requires.
    nc = bacc.Bacc()
    d14 = nc.dram_tensor("a14", [14, 264], F32, kind="ExternalInput")
    d128 = nc.dram_tensor("a128", [128, 487], F32, kind="ExternalInput")
    d33 = nc.dram_tensor("a33", [33, 528], F32, kind="ExternalInput")
    d9 = nc.dram_tensor("a9", [9, 73], F32, kind="ExternalInput")
    dout = nc.dram_tensor("out", [1, 8], F32, kind="ExternalOutput")

    def ap(base, offset_delta, dims):
        """Custom AP on the same tensor as `base`, free dims replaced."""
        return bass.AP(tensor=base.tensor, offset=base.offset + offset_delta,
                       ap=[base.ap[0]] + dims)

    F32R_ = mybir.dt.float32r

    def r32(x):
        # producers feeding float32r matmuls must emit float32r
        return x.bitcast(F32R_)

    TileContext._drain_and_barrier = _slim_drain_and_barrier
    with TileContext(nc) as tc:
        with (
            tc.tile_pool(name="sb", bufs=1) as sb,
            tc.tile_pool(name="ps", bufs=8, space="PSUM") as ps,
        ):
            # ---- input DMAs, spread over the two HWDGE queues (SP + ACT);
            # each engine's queue runs serially, so order by need time ----
            a14 = sb.tile([14, 264], F32, tag="a14")
            nc.sync.dma_start(out=r32(a14), in_=r32(d14[:, :]))
            a128 = sb.tile([128, 487], F32, tag="a128")
            nc.scalar.dma_start(out=r32(a128), in_=r32(d128[:, :]))
            a33 = sb.tile([33, 528], F32, tag="a33")
            nc.sync.dma_start(out=r32(a33), in_=r32(d33[:, :]))
            a9 = sb.tile([9, 73], F32, tag="a9")
            nc.scalar.dma_start(out=r32(a9), in_=r32(d9[:, :]))


            # warm ACT's activation table off the critical path (first
            # ACTIVATE pays a ~1.8us table load; do it on scratch now)
            warm = sb.tile([1, 1], F32, tag="warm")
            nc.scalar.mul(warm, warm, 0.0)

            F32R = mybir.dt.float32r

            def mm(out, lhsT, rhs, start, stop, r=True):
                # float32r: single-pass PE fp32 (vs 2 half-speed passes)
                if r:
                    lhsT = lhsT.bitcast(F32R)
                    rhs = rhs.bitcast(F32R)
                nc.tensor.matmul(out, lhsT=lhsT, rhs=rhs, start=start, stop=stop)

            ts = nc.vector.tensor_scalar
            cp = nc.vector.tensor_copy
            cpa = nc.scalar.copy  # prep copies go on ACT to keep DVE clear

            # ---- stage A: demand MLP 14 -> 256 -> 32 ----
            p1a = ps.tile([128, 8], F32, tag="ps")
            mm(p1a, lhsT=a14[0:14, 8:136], rhs=a14[0:14, 0:8], start=True, stop=True)
            p1b = ps.tile([128, 8], F32, tag="ps")
            mm(p1b, lhsT=a14[0:14, 136:264], rhs=a14[0:14, 0:8], start=True, stop=True)
            h1a = sb.tile([128, 8], F32, tag="h1a")
            ts(r32(h1a), p1a, a128[:, 224:225], 0.0, op0=ADD, op1=MAX)
            h1b = sb.tile([128, 8], F32, tag="h1b")
            nc.scalar.activation(r32(h1b), p1b, mybir.ActivationFunctionType.Relu, bias=a128[:, 225:226])

            p2 = ps.tile([32, 8], F32, tag="ps")
            mm(p2, lhsT=a128[0:128, 0:32], rhs=h1a, start=True, stop=False)
            mm(p2, lhsT=a128[0:128, 32:64], rhs=h1b, start=False, stop=True)

            # h2dup (33, 16): cols 0:8 = relu(h2)[:, piA], cols 8:16 = [:, piB],
            # row 32 = ones
            h2dup = sb.tile([33, 16], F32, tag="h2dup")
            ts(r32(h2dup[32:33, 0:16]), a14[0:1, 0:16], 0.0, 1.0,
               op0=mybir.AluOpType.mult, op1=ADD)
            # top gather [0,0,1,1,2,2,3,3]: in [[1,4],[0,2]], out [[2,4],[1,2]]
            ts(r32(ap(h2dup[0:32, 0:8], 0, [[2, 4], [1, 2]])),
               ap(p2[0:32, 0:8], 0, [[1, 4], [0, 2]]),
               a33[0:32, 4:5], 0.0, op0=ADD, op1=MAX)
            # bot gather [4,5,4,5,6,7,6,7]: in off4 [[2,2],[0,2],[1,2]],
            # out off8 [[4,2],[2,2],[1,2]]
            ts(r32(ap(h2dup[0:32, 0:8], 8, [[4, 2], [2, 2], [1, 2]])),
               ap(p2[0:32, 0:8], 4, [[2, 2], [0, 2], [1, 2]]),
               a33[0:32, 4:5], 0.0, op0=ADD, op1=MAX)

            # ---- fused demand-L3 + pair gather + coupled-L1 ----
            pB1 = ps.tile([128, 8], F32, tag="ps")
            mm(pB1, lhsT=a33[0:33, 16:144], rhs=h2dup[0:33, 0:8], start=True, stop=False)
            mm(pB1, lhsT=a33[0:33, 144:272], rhs=h2dup[0:33, 8:16], start=False, stop=True)
            hc1 = sb.tile([128, 8], F32, tag="hc1")
            ts(r32(hc1), pB1, a128[:, 228:229], 0.0, op0=ADD, op1=MAX)

            pB2 = ps.tile([32, 8], F32, tag="ps")
            mm(pB2, lhsT=a128[0:128, 192:224], rhs=hc1, start=True, stop=True)
            hc2e = sb.tile([33, 8], F32, tag="hc2e")
            ts(r32(hc2e[32:33, :]), a14[0:1, 0:8], 0.0, 1.0,
               op0=mybir.AluOpType.mult, op1=ADD)
            ts(r32(hc2e[0:32, :]), pB2, a33[0:32, 6:7], 0.0, op0=ADD, op1=MAX)

            # ---- dp / dpT and the D tensor (8, 56) ----
            pdp = ps.tile([8, 4], F32, tag="ps")
            mm(pdp, lhsT=hc2e[0:33, :], rhs=a33[0:33, 0:4], start=True, stop=True)
            pdpT = ps.tile([4, 8], F32, tag="ps")
            mm(pdpT, lhsT=a33[0:33, 0:4], rhs=hc2e[0:33, :], start=True, stop=True)

            dp_s = sb.tile([8, 4], F32, tag="dp_s")
            cp(out=r32(dp_s), in_=pdp)
            # D as (36, 56): top 4 channels at rows 0:4, bottom 4 at rows
            # 32:36 (legal DVE write base), rows 4:32 zeroed; the matching
            # WD1T36 weight rows are zero-padded on the host.
            D36 = sb.tile([36, 56], F32, tag="D36")
            ts(r32(D36[0:32, :]), a128[0:32, 0:56], 0.0, None,
               op0=mybir.AluOpType.mult)
            cp(out=r32(ap(D36[0:4, 0:56], 0, [[7, 8], [1, 7]])),
               in_=ap(pdpT[0:4, 0:8], 0, [[1, 8], [0, 7]]))
            pDb = ps.tile([4, 56], F32, tag="ps")
            mm(pDb, lhsT=dp_s, rhs=a9[0:8, 7:63], start=True, stop=True)
            cp(out=r32(D36[32:36, :]), in_=pDb)

            # ---- Conv_D: 8 -> 256 -> 32 ----
            pD1a = ps.tile([128, 56], F32, tag="ps")
            mm(pD1a, lhsT=a128[0:36, 231:359], rhs=D36, start=True, stop=True)
            pD1b = ps.tile([128, 56], F32, tag="ps")
            mm(pD1b, lhsT=a128[0:36, 359:487], rhs=D36, start=True, stop=True)
            hd1a = sb.tile([128, 56], F32, tag="hd1a")
            ts(r32(hd1a), pD1a, a128[:, 226:227], 0.0, op0=ADD, op1=MAX)
            hd1b = sb.tile([128, 56], F32, tag="hd1b")
            nc.scalar.activation(r32(hd1b), pD1b, mybir.ActivationFunctionType.Relu, bias=a128[:, 227:228])

            pD2 = ps.tile([32, 56], F32, tag="ps")
            mm(pD2, lhsT=a128[0:128, 64:96], rhs=hd1a, start=True, stop=False)
            mm(pD2, lhsT=a128[0:128, 96:128], rhs=hd1b, start=False, stop=True)
            hd2 = sb.tile([32, 56], F32, tag="hd2")
            ts(r32(hd2), pD2, a33[0:32, 5:6], 0.0, op0=ADD, op1=MAX)

            # ---- fused Conv_D-L3 + Conv_C-L1: 32 -> 256 ----
            pC1a = ps.tile([128, 56], F32, tag="ps")
            mm(pC1a, lhsT=a33[0:32, 272:400], rhs=hd2, start=True, stop=True)
            pC1b = ps.tile([128, 56], F32, tag="ps")
            mm(pC1b, lhsT=a33[0:32, 400:528], rhs=hd2, start=True, stop=True)
            hq1a = sb.tile([128, 56], F32, tag="hq1a")
            ts(r32(hq1a), pC1a, a128[:, 229:230], 0.0, op0=ADD, op1=MAX)
            hq1b = sb.tile([128, 56], F32, tag="hq1b")
            nc.scalar.activation(r32(hq1b), pC1b, mybir.ActivationFunctionType.Relu, bias=a128[:, 230:231])

            # ---- Conv_C: 256 -> 32 -> 8 ----
            pC2 = ps.tile([32, 56], F32, tag="ps")
            mm(pC2, lhsT=a128[0:128, 128:160], rhs=hq1a, start=True, stop=False)
            mm(pC2, lhsT=a128[0:128, 160:192], rhs=hq1b, start=False, stop=True)
            hq2 = sb.tile([32, 56], F32, tag="hq2")
            ts(r32(hq2), pC2, a33[0:32, 7:8], 0.0, op0=ADD, op1=MAX)

            pC3 = ps.tile([8, 56], F32, tag="ps")
            mm(pC3, lhsT=a33[0:32, 8:16], rhs=hq2, start=True, stop=True)
            hf = sb.tile([8, 56], F32, tag="hf")
            ts(r32(hf[0:8, :]), pC3, a9[0:8, 63:64], 0.0, op0=ADD, op1=MAX)

            # ---- final (1,7) conv: bias matmul (no data deps, issues early)
            # then 7 accumulating matmuls over the w offsets ----
            pout = ps.tile([1, 8], F32, tag="ps")
            mm(pout, lhsT=a9[0:1, 64:65], rhs=a9[0:1, 65:73],
               start=True, stop=False)
            for w in range(7):
                mm(pout, lhsT=a9[0:8, w:w + 1],
                   rhs=ap(hf[0:8, 0:8], w, [[7, 8]]),
                   start=False, stop=(w == 6))
            out_s = sb.tile([1, 8], F32, tag="out_s")
            cp(out=out_s, in_=pout)
            nc.sync.dma_start(out=dout[:, :], in_=out_s)

    nc.compile()
    return nc


def _get_nc():
    if "nc" not in _NC_CACHE:
        _NC_CACHE["nc"] = _build_nc()
    return _NC_CACHE["nc"]


# ---------------------------------------------------------------------------
# public entry points
# ---------------------------------------------------------------------------
def _unpermute(raw8):
    out = np.empty(8, np.float32)
    out[_ORIGROW] = raw8
    return out


def run_spmd(inputs, trace=False):
    """Run on the 8 NeuronCores; returns (output(8,), BassKernelResults)."""
    from concourse.bass_utils import run_bass_kernel_spmd
    packed = _host_pack(inputs)
    nc = _get_nc()
    core_ids = list(range(8))
    in_maps = [dict(packed) for _ in core_ids]
    res = run_bass_kernel_spmd(nc, in_maps, core_ids, trace=trace)
    out = _unpermute(np.asarray(res.results[0]["out"], np.float32)[0])
    return out, res


def run_sim(inputs):
    """CoreSim path (no hardware) for debugging."""
    from concourse import bass_interp
    packed = _host_pack(inputs)
    nc = _build_nc()
    sim = bass_interp.CoreSim(nc)
    for k, v in packed.items():
        sim.tensor(k)[:] = v
    sim.simulate()
    return _unpermute(np.asarray(sim.tensor("out"), np.float32)[0])


def kernel(**inputs):
    out, _ = run_spmd(inputs, trace=False)
    return out


# revision 39
# speedup vs baseline: 1.0143x; 1.0143x over previous
"""Trainium2 Bass kernel for nn_FRAP_47047071761018.

FRAP forward pass (demand MLP -> coupled-demand MLP -> D-tensor -> two
1x1-conv stacks -> (1,7) conv). Batch=1, tiny dims => latency problem.
Strategy: replicate the whole computation on all 8 cores (no useful
tensor sharding), read the result from core 0.

Device dataflow (float32r, features-on-partitions orientation):
  - every Linear is out = lhsT.T @ rhs on the PE with K on partitions;
    operands are float32r (single-pass PE fp32, ~2x the fp32 rate)
  - relu(x+b) fused into one DVE tensor_scalar (op0=add per-partition
    bias, op1=max 0); the second relu of 128-row pairs runs on ACT
  - host folds (weight preprocessing): demand-L3 + pair-gather +
    coupled-L1 -> FaT/FbT = (c_w1[:, :4] @ [d_w3|d_b3])^T, and
    Conv_D-L3 + Conv_C-L1 -> WeffT = (cC_w1[:, :2] @ dD_w3)^T with
    beff = cC_w1[:, :2] @ dD_b3 + cC_b1
  - D tensor as a (36, 56) tile: top channels at rows 0:4, bottom at
    rows 32:36 (legal DVE write bases), zero rows between, matched by
    zero-padded WD1T36 weights -> Conv_D L1 is 2 matmuls
  - final (1,7) conv = 7 accumulating matmuls with stride-7 rhs APs,
    bias via an extra ones-partition row
  - inputs arrive as 4 packed DRAM arrays over both HWDGE queues
    (SP + ACT), ordered by need time; the Tile exit tail is slimmed to
    just the completion drain (the Bass preamble re-clears semaphores
    at the start of every execution)
Pair rows are processed in a permuted order (PI) chosen so both lane
gathers are strided APs; the host unpermutes the 8 outputs at the end.
"""

import numpy as np

# ---------------------------------------------------------------------------
# static structure (from the FRAP definition; hardcoded, no file reads)
# ---------------------------------------------------------------------------
# reference PAIRS rows:
_PAIRS = [(0, 4), (1, 5), (2, 6), (3, 7), (0, 5), (1, 4), (2, 7), (3, 6)]
# our processing order: both column projections are strided-AP sequences
_PI_PAIRS = [(0, 4), (0, 5), (1, 4), (1, 5), (2, 6), (2, 7), (3, 6), (3, 7)]
_ORIGROW = [_PAIRS.index(p) for p in _PI_PAIRS]          # p' -> original row
_POS_OF_ORIG = np.argsort(_ORIGROW)                      # original row -> p'


def _build_bbot():
    """Bbot[q, p'*7+k] = 1 iff ORIGROW[q] == j(ORIGROW[p'], k), j=k+(k>=i)."""
    B = np.zeros((8, 56), np.float32)
    for pp in range(8):
        i = _ORIGROW[pp]
        for k in range(7):
            jj = k + (1 if k >= i else 0)
            B[_POS_OF_ORIG[jj], pp * 7 + k] = 1.0
    return B


def _host_pack(inp):
    """Marshal kernel inputs into the 7 packed DRAM arrays the device
    program expects (transposes / concats / 0-1 selection constants)."""
    f = lambda x: np.ascontiguousarray(np.asarray(x, np.float32))
    # --- a14 (14, 264): XT | W1dT ---
    XT = np.concatenate([
        np.broadcast_to(f(inp["i_position"])[0, 0], (1, 8)),
        np.broadcast_to(f(inp["j_position"])[0, 0], (1, 8)),
        f(inp["wtime_mu"]), f(inp["wtime_sigma"]),
        f(inp["wtime_max"]), f(inp["waiting"]),
        np.broadcast_to(f(inp["phase"]).T, (8, 8)),
    ], axis=0)
    a14 = np.concatenate([XT, f(inp["d_w1"]).T], axis=1)          # (14, 264)

    # --- a128 (128, 229): W2dT(64) WD2T(64) WC2T(64) Wc2T(32) b1(2) bD1(2) bc1(1)
    w2t = f(inp["d_w2"]).T
    wd2t = f(inp["dD_w2"]).T
    wc2t = f(inp["cC_w2"]).T
    beff = (f(inp["cC_w1"])[:, 0:2] @ f(inp["dD_b3"]) + f(inp["cC_b1"]))
    a128 = np.concatenate([
        w2t[:128], w2t[128:], wd2t[:128], wd2t[128:], wc2t[:128], wc2t[128:],
        f(inp["c_w2"]).T,
        f(inp["d_b1"]).reshape(2, 128).T, f(inp["dD_b1"]).reshape(2, 128).T,
        f(inp["c_b1"]).reshape(128, 1),
        beff.reshape(2, 128).T,
    ], axis=1)                                                    # (128, 231)
    wd1t = f(inp["dD_w1"]).T                                      # (8, 256)
    wd1t36 = np.zeros((128, 256), np.float32)
    wd1t36[0:4] = wd1t[0:4]
    wd1t36[32:36] = wd1t[4:8]
    a128 = np.concatenate([a128, wd1t36], axis=1)                 # (128, 487)

    # --- a33 (33, 528): [Wc3Te(4) b2 bD2 bc2 bc2C WcC3T(8)] | FaT(128) | FbT(128)
    #     | WeffT(256)  (host-folded weight products)
    a33 = np.zeros((33, 528), np.float32)
    a33[0:32, 0:4] = f(inp["c_w3"]).T[0:32]
    a33[32, 0:4] = f(inp["c_b3"])
    a33[0:32, 4] = f(inp["d_b2"])
    a33[0:32, 5] = f(inp["dD_b2"])
    a33[0:32, 6] = f(inp["c_b2"])
    a33[0:32, 7] = f(inp["cC_b2"])
    a33[0:32, 8:16] = f(inp["cC_w3"]).T
    w3aug = np.concatenate([f(inp["d_w3"]), f(inp["d_b3"])[:, None]], axis=1)  # (4,33)
    wc1 = f(inp["c_w1"])                                          # (128, 8)
    a33[0:33, 16:144] = (wc1[:, 0:4] @ w3aug).T
    a33[0:33, 144:272] = (wc1[:, 4:8] @ w3aug).T
    a33[0:32, 272:528] = (f(inp["cC_w1"])[:, 0:2] @ f(inp["dD_w3"])).T

    # --- a9 (9, 73): W4e 0:7 | Bbot 7:63 | bc3C 63 | b4 [0,64] | ones [0,65:73]
    a9 = np.zeros((9, 73), np.float32)
    a9[0:8, 0:7] = f(inp["cC_w4"])
    a9[0:8, 7:63] = _build_bbot()
    a9[0:8, 63] = f(inp["cC_b3"])
    a9[0, 64] = f(inp["cC_b4"])[0]
    a9[0, 65:73] = 1.0

    return {
        "a14": np.ascontiguousarray(a14), "a128": np.ascontiguousarray(a128),
        "a33": a33, "a9": a9,
    }


# ---------------------------------------------------------------------------
# bass program
# ---------------------------------------------------------------------------
_NC_CACHE = {}


def _build_nc():
    import concourse.bass as bass
    import concourse.bacc as bacc
    import concourse.mybir as mybir
    import concourse.tile as tile_mod
    from concourse.tile import TileContext
    from concourse.vector_clock import ScopedClock

    # Slim kernel tail: keep the completion drain (waits for the out-DMA
    # queue, guaranteeing the output is in DRAM before the NEFF retires)
    # but skip the two all-engine barriers + per-sem zeroing. Safe here
    # because the Bass preamble dma_reset/sem_clears the full kernel sem
    # range at the START of every execution, so re-exec state is clean.
    def _slim_drain_and_barrier(self, tick_clock, wait_clock):
        drain_inst = self.nc.sync.drain()
        wait_clock.add_sem_waits(
            drain_inst.ins, ScopedClock({None: tick_clock.global_clock})
        )
        popped = self.nc._tile_sem_poison_stack.pop()
        assert popped is self._sem_poison

    F32 = mybir.dt.float32
    ADD = mybir.AluOpType.add
    MAX = mybir.AluOpType.max

    # Bacc (not raw Bass): its compile() runs move_matmul_waits_to_ldweights
    # + generate_event_semaphores, which split multi-wait instructions into
    # the 1-wait form TRN2 walrus codegen requires.
    nc = bacc.Bacc()
    d14 = nc.dram_tensor("a14", [14, 264], F32, kind="ExternalInput")
    d128 = nc.dram_tensor("a128", [128, 487], F32, kind="ExternalInput")
    d33 = nc.dram_tensor("a33", [33, 528], F32, kind="ExternalInput")
    d9 = nc.dram_tensor("a9", [9, 73], F32, kind="ExternalInput")
    dout = nc.dram_tensor("out", [1, 8], F32, kind="ExternalOutput")

    def ap(base, offset_delta, dims):
        """Custom AP on the same tensor as `base`, free dims replaced."""
        return bass.AP(tensor=base.tensor, offset=base.offset + offset_delta,
                       ap=[base.ap[0]] + dims)

    F32R_ = mybir.dt.float32r

    def r32(x):
        # producers feeding float32r matmuls must emit float32r
        return x.bitcast(F32R_)

    TileContext._drain_and_barrier = _slim_drain_and_barrier
    with TileContext(nc) as tc:
        with (
            tc.tile_pool(name="sb", bufs=1) as sb,
            tc.tile_pool(name="ps", bufs=8, space="PSUM") as ps,
        ):
            # ---- input DMAs, spread over the two HWDGE queues (SP + ACT);
            # each engine's queue runs serially, so order by need time ----
            # a14 gates the whole chain; keep it ALONE on the sync queue —
            # completion sems only become visible once a queue drains, so a
            # second sync-queue DMA would push the chain start out with it.
            a14 = sb.tile([14, 264], F32, tag="a14")
            nc.sync.dma_start(out=r32(a14), in_=r32(d14[:, :]))
            a128 = sb.tile([128, 487], F32, tag="a128")
            nc.scalar.dma_start(out=r32(a128), in_=r32(d128[:, :]))
            a33 = sb.tile([33, 528], F32, tag="a33")
            nc.scalar.dma_start(out=r32(a33), in_=r32(d33[:, :]))
            a9 = sb.tile([9, 73], F32, tag="a9")
            nc.scalar.dma_start(out=r32(a9), in_=r32(d9[:, :]))


            # warm ACT's activation table off the critical path (first
            # ACTIVATE pays a ~1.8us table load; do it on scratch now)
            warm = sb.tile([1, 1], F32, tag="warm")
            nc.scalar.mul(warm, warm, 0.0)

            F32R = mybir.dt.float32r

            def mm(out, lhsT, rhs, start, stop, r=True):
                # float32r: single-pass PE fp32 (vs 2 half-speed passes)
                if r:
                    lhsT = lhsT.bitcast(F32R)
                    rhs = rhs.bitcast(F32R)
                nc.tensor.matmul(out, lhsT=lhsT, rhs=rhs, start=start, stop=stop)

            ts = nc.vector.tensor_scalar
            cp = nc.vector.tensor_copy
            cpa = nc.scalar.copy  # prep copies go on ACT to keep DVE clear

            # ---- stage A: demand MLP 14 -> 256 -> 32 ----
            p1a = ps.tile([128, 8], F32, tag="ps")
            mm(p1a, lhsT=a14[0:14, 8:136], rhs=a14[0:14, 0:8], start=True, stop=True)
            p1b = ps.tile([128, 8], F32, tag="ps")
            mm(p1b, lhsT=a14[0:14, 136:264], rhs=a14[0:14, 0:8], start=True, stop=True)
            h1a = sb.tile([128, 8], F32, tag="h1a")
            ts(r32(h1a), p1a, a128[:, 224:225], 0.0, op0=ADD, op1=MAX)
            h1b = sb.tile([128, 8], F32, tag="h1b")
            nc.scalar.activation(r32(h1b), p1b, mybir.ActivationFunctionType.Relu, bias=a128[:, 225:226])

            p2 = ps.tile([32, 8], F32, tag="ps")
            mm(p2, lhsT=a128[0:128, 0:32], rhs=h1a, start=True, stop=False)
            mm(p2, lhsT=a128[0:128, 32:64], rhs=h1b, start=False, stop=True)

            # h2dup (33, 16): cols 0:8 = relu(h2)[:, piA], cols 8:16 = [:, piB],
            # row 32 = ones
            h2dup = sb.tile([33, 16], F32, tag="h2dup")
            ts(r32(h2dup[32:33, 0:16]), a14[0:1, 0:16], 0.0, 1.0,
               op0=mybir.AluOpType.mult, op1=ADD)
            # top gather [0,0,1,1,2,2,3,3]: in [[1,4],[0,2]], out [[2,4],[1,2]]
            ts(r32(ap(h2dup[0:32, 0:8], 0, [[2, 4], [1, 2]])),
               ap(p2[0:32, 0:8], 0, [[1, 4], [0, 2]]),
               a33[0:32, 4:5], 0.0, op0=ADD, op1=MAX)
            # bot gather [4,5,4,5,6,7,6,7]: in off4 [[2,2],[0,2],[1,2]],
            # out off8 [[4,2],[2,2],[1,2]]
            ts(r32(ap(h2dup[0:32, 0:8], 8, [[4, 2], [2, 2], [1, 2]])),
               ap(p2[0:32, 0:8], 4, [[2, 2], [0, 2], [1, 2]]),
               a33[0:32, 4:5], 0.0, op0=ADD, op1=MAX)

            # ---- fused demand-L3 + pair gather + coupled-L1 ----
            pB1 = ps.tile([128, 8], F32, tag="ps")
            mm(pB1, lhsT=a33[0:33, 16:144], rhs=h2dup[0:33, 0:8], start=True, stop=False)
            mm(pB1, lhsT=a33[0:33, 144:272], rhs=h2dup[0:33, 8:16], start=False, stop=True)
            hc1 = sb.tile([128, 8], F32, tag="hc1")
            ts(r32(hc1), pB1, a128[:, 228:229], 0.0, op0=ADD, op1=MAX)

            pB2 = ps.tile([32, 8], F32, tag="ps")
            mm(pB2, lhsT=a128[0:128, 192:224], rhs=hc1, start=True, stop=True)
            hc2e = sb.tile([33, 8], F32, tag="hc2e")
            ts(r32(hc2e[32:33, :]), a14[0:1, 0:8], 0.0, 1.0,
               op0=mybir.AluOpType.mult, op1=ADD)
            ts(r32(hc2e[0:32, :]), pB2, a33[0:32, 6:7], 0.0, op0=ADD, op1=MAX)

            # ---- dp / dpT and the D tensor (8, 56) ----
            pdp = ps.tile([8, 4], F32, tag="ps")
            mm(pdp, lhsT=hc2e[0:33, :], rhs=a33[0:33, 0:4], start=True, stop=True)
            pdpT = ps.tile([4, 8], F32, tag="ps")
            mm(pdpT, lhsT=a33[0:33, 0:4], rhs=hc2e[0:33, :], start=True, stop=True)

            dp_s = sb.tile([8, 4], F32, tag="dp_s")
            cp(out=r32(dp_s), in_=pdp)
            # D as (36, 56): top 4 channels at rows 0:4, bottom 4 at rows
            # 32:36 (legal DVE write base), rows 4:32 zeroed; the matching
            # WD1T36 weight rows are zero-padded on the host.
            D36 = sb.tile([36, 56], F32, tag="D36")
            ts(r32(D36[0:32, :]), a128[0:32, 0:56], 0.0, None,
               op0=mybir.AluOpType.mult)
            cp(out=r32(ap(D36[0:4, 0:56], 0, [[7, 8], [1, 7]])),
               in_=ap(pdpT[0:4, 0:8], 0, [[1, 8], [0, 7]]))
            pDb = ps.tile([4, 56], F32, tag="ps")
            mm(pDb, lhsT=dp_s, rhs=a9[0:8, 7:63], start=True, stop=True)
            cp(out=r32(D36[32:36, :]), in_=pDb)

            # ---- Conv_D: 8 -> 256 -> 32 ----
            pD1a = ps.tile([128, 56], F32, tag="ps")
            mm(pD1a, lhsT=a128[0:36, 231:359], rhs=D36, start=True, stop=True)
            pD1b = ps.tile([128, 56], F32, tag="ps")
            mm(pD1b, lhsT=a128[0:36, 359:487], rhs=D36, start=True, stop=True)
            hd1a = sb.tile([128, 56], F32, tag="hd1a")
            ts(r32(hd1a), pD1a, a128[:, 226:227], 0.0, op0=ADD, op1=MAX)
            hd1b = sb.tile([128, 56], F32, tag="hd1b")
            nc.scalar.activation(r32(hd1b), pD1b, mybir.ActivationFunctionType.Relu, bias=a128[:, 227:228])

            pD2 = ps.tile([32, 56], F32, tag="ps")
            mm(pD2, lhsT=a128[0:128, 64:96], rhs=hd1a, start=True, stop=False)
            mm(pD2, lhsT=a128[0:128, 96:128], rhs=hd1b, start=False, stop=True)
            hd2 = sb.tile([32, 56], F32, tag="hd2")
            ts(r32(hd2), pD2, a33[0:32, 5:6], 0.0, op0=ADD, op1=MAX)

            # ---- fused Conv_D-L3 + Conv_C-L1: 32 -> 256 ----
            pC1a = ps.tile([128, 56], F32, tag="ps")
            mm(pC1a, lhsT=a33[0:32, 272:400], rhs=hd2, start=True, stop=True)
            pC1b = ps.tile([128, 56], F32, tag="ps")
            mm(pC1b, lhsT=a33[0:32, 400:528], rhs=hd2, start=True, stop=True)
            hq1a = sb.tile([128, 56], F32, tag="hq1a")
            ts(r32(hq1a), pC1a, a128[:, 229:230], 0.0, op0=ADD, op1=MAX)
            hq1b = sb.tile([128, 56], F32, tag="hq1b")
            nc.scalar.activation(r32(hq1b), pC1b, mybir.ActivationFunctionType.Relu, bias=a128[:, 230:231])

            # ---- Conv_C: 256 -> 32 -> 8 ----
            pC2 = ps.tile([32, 56], F32, tag="ps")
            mm(pC2, lhsT=a128[0:128, 128:160], rhs=hq1a, start=True, stop=False)
            mm(pC2, lhsT=a128[0:128, 160:192], rhs=hq1b, start=False, stop=True)
            hq2 = sb.tile([32, 56], F32, tag="hq2")
            ts(r32(hq2), pC2, a33[0:32, 7:8], 0.0, op0=ADD, op1=MAX)

            pC3 = ps.tile([8, 56], F32, tag="ps")
            mm(pC3, lhsT=a33[0:32, 8:16], rhs=hq2, start=True, stop=True)
            hf = sb.tile([8, 56], F32, tag="hf")
            ts(r32(hf[0:8, :]), pC3, a9[0:8, 63:64], 0.0, op0=ADD, op1=MAX)

            # ---- final (1,7) conv: bias matmul (no data deps, issues early)
            # then 7 accumulating matmuls over the w offsets ----
            pout = ps.tile([1, 8], F32, tag="ps")
            mm(pout, lhsT=a9[0:1, 64:65], rhs=a9[0:1, 65:73],
               start=True, stop=False)
            for w in range(7):
                mm(pout, lhsT=a9[0:8, w:w + 1],
                   rhs=ap(hf[0:8, 0:8], w, [[7, 8]]),
                   start=False, stop=(w == 6))
            out_s = sb.tile([1, 8], F32, tag="out_s")
            cp(out=out_s, in_=pout)
            nc.sync.dma_start(out=dout[:, :], in_=out_s)

    nc.compile()
    return nc


def _get_nc():
    if "nc" not in _NC_CACHE:
        _NC_CACHE["nc"] = _build_nc()
    return _NC_CACHE["nc"]


# ---------------------------------------------------------------------------
# public entry points
# ---------------------------------------------------------------------------
def _unpermute(raw8):
    out = np.empty(8, np.float32)
    out[_ORIGROW] = raw8
    return out


def run_spmd(inputs, trace=False):
    """Run on the 8 NeuronCores; returns (output(8,), BassKernelResults)."""
    from concourse.bass_utils import run_bass_kernel_spmd
    packed = _host_pack(inputs)
    nc = _get_nc()
    core_ids = list(range(8))
    in_maps = [dict(packed) for _ in core_ids]
    res = run_bass_kernel_spmd(nc, in_maps, core_ids, trace=trace)
    out = _unpermute(np.asarray(res.results[0]["out"], np.float32)[0])
    return out, res


def run_sim(inputs):
    """CoreSim path (no hardware) for debugging."""
    from concourse import bass_interp
    packed = _host_pack(inputs)
    nc = _build_nc()
    sim = bass_interp.CoreSim(nc)
    for k, v in packed.items():
        sim.tensor(k)[:] = v
    sim.simulate()
    return _unpermute(np.asarray(sim.tensor("out"), np.float32)[0])


def kernel(**inputs):
    out, _ = run_spmd(inputs, trace=False)
    return out


# revision 40
# speedup vs baseline: 1.0292x; 1.0147x over previous
"""Trainium2 Bass kernel for nn_FRAP_47047071761018.

FRAP forward pass (demand MLP -> coupled-demand MLP -> D-tensor -> two
1x1-conv stacks -> (1,7) conv). Batch=1, tiny dims => latency problem.
Strategy: replicate the whole computation on all 8 cores (no useful
tensor sharding), read the result from core 0.

Device dataflow (float32r, features-on-partitions orientation):
  - every Linear is out = lhsT.T @ rhs on the PE with K on partitions;
    operands are float32r (single-pass PE fp32, ~2x the fp32 rate)
  - relu(x+b) fused into one DVE tensor_scalar (op0=add per-partition
    bias, op1=max 0); the second relu of 128-row pairs runs on ACT
  - host folds (weight preprocessing): demand-L3 + pair-gather +
    coupled-L1 -> FaT/FbT = (c_w1[:, :4] @ [d_w3|d_b3])^T, and
    Conv_D-L3 + Conv_C-L1 -> WeffT = (cC_w1[:, :2] @ dD_w3)^T with
    beff = cC_w1[:, :2] @ dD_b3 + cC_b1
  - D tensor as a (36, 56) tile: top channels at rows 0:4, bottom at
    rows 32:36 (legal DVE write bases), zero rows between, matched by
    zero-padded WD1T36 weights -> Conv_D L1 is 2 matmuls
  - final (1,7) conv = 7 accumulating matmuls with stride-7 rhs APs,
    bias via an extra ones-partition row
  - inputs arrive as 4 packed DRAM arrays over both HWDGE queues
    (SP + ACT), ordered by need time; the Tile exit tail is slimmed to
    just the completion drain (the Bass preamble re-clears semaphores
    at the start of every execution)
Pair rows are processed in a permuted order (PI) chosen so both lane
gathers are strided APs; the host unpermutes the 8 outputs at the end.
"""

import numpy as np

# ---------------------------------------------------------------------------
# static structure (from the FRAP definition; hardcoded, no file reads)
# ---------------------------------------------------------------------------
# reference PAIRS rows:
_PAIRS = [(0, 4), (1, 5), (2, 6), (3, 7), (0, 5), (1, 4), (2, 7), (3, 6)]
# our processing order: both column projections are strided-AP sequences
_PI_PAIRS = [(0, 4), (0, 5), (1, 4), (1, 5), (2, 6), (2, 7), (3, 6), (3, 7)]
_ORIGROW = [_PAIRS.index(p) for p in _PI_PAIRS]          # p' -> original row
_POS_OF_ORIG = np.argsort(_ORIGROW)                      # original row -> p'


def _build_bbot():
    """Bbot[q, p'*7+k] = 1 iff ORIGROW[q] == j(ORIGROW[p'], k), j=k+(k>=i)."""
    B = np.zeros((8, 56), np.float32)
    for pp in range(8):
        i = _ORIGROW[pp]
        for k in range(7):
            jj = k + (1 if k >= i else 0)
            B[_POS_OF_ORIG[jj], pp * 7 + k] = 1.0
    return B


def _host_pack(inp):
    """Marshal kernel inputs into the 7 packed DRAM arrays the device
    program expects (transposes / concats / 0-1 selection constants)."""
    f = lambda x: np.ascontiguousarray(np.asarray(x, np.float32))
    # --- a14 (14, 264): XT | W1dT ---
    XT = np.concatenate([
        np.broadcast_to(f(inp["i_position"])[0, 0], (1, 8)),
        np.broadcast_to(f(inp["j_position"])[0, 0], (1, 8)),
        f(inp["wtime_mu"]), f(inp["wtime_sigma"]),
        f(inp["wtime_max"]), f(inp["waiting"]),
        np.broadcast_to(f(inp["phase"]).T, (8, 8)),
    ], axis=0)
    a14 = np.concatenate([XT, f(inp["d_w1"]).T], axis=1)          # (14, 264)

    # --- a128 (128, 229): W2dT(64) WD2T(64) WC2T(64) Wc2T(32) b1(2) bD1(2) bc1(1)
    w2t = f(inp["d_w2"]).T
    wd2t = f(inp["dD_w2"]).T
    wc2t = f(inp["cC_w2"]).T
    beff = (f(inp["cC_w1"])[:, 0:2] @ f(inp["dD_b3"]) + f(inp["cC_b1"]))
    a128 = np.concatenate([
        w2t[:128], w2t[128:], wd2t[:128], wd2t[128:], wc2t[:128], wc2t[128:],
        f(inp["c_w2"]).T,
        f(inp["d_b1"]).reshape(2, 128).T, f(inp["dD_b1"]).reshape(2, 128).T,
        f(inp["c_b1"]).reshape(128, 1),
        beff.reshape(2, 128).T,
    ], axis=1)                                                    # (128, 231)
    wd1t = f(inp["dD_w1"]).T                                      # (8, 256)
    wd1t36 = np.zeros((128, 256), np.float32)
    wd1t36[0:4] = wd1t[0:4]
    wd1t36[32:36] = wd1t[4:8]
    a128 = np.concatenate([a128, wd1t36], axis=1)                 # (128, 487)

    # --- a33 (33, 528): [Wc3Te(4) b2 bD2 bc2 bc2C WcC3T(8)] | FaT(128) | FbT(128)
    #     | WeffT(256)  (host-folded weight products)
    a33 = np.zeros((33, 528), np.float32)
    a33[0:32, 0:4] = f(inp["c_w3"]).T[0:32]
    a33[32, 0:4] = f(inp["c_b3"])
    a33[0:32, 4] = f(inp["d_b2"])
    a33[0:32, 5] = f(inp["dD_b2"])
    a33[0:32, 6] = f(inp["c_b2"])
    a33[0:32, 7] = f(inp["cC_b2"])
    a33[0:32, 8:16] = f(inp["cC_w3"]).T
    w3aug = np.concatenate([f(inp["d_w3"]), f(inp["d_b3"])[:, None]], axis=1)  # (4,33)
    wc1 = f(inp["c_w1"])                                          # (128, 8)
    a33[0:33, 16:144] = (wc1[:, 0:4] @ w3aug).T
    a33[0:33, 144:272] = (wc1[:, 4:8] @ w3aug).T
    a33[0:32, 272:528] = (f(inp["cC_w1"])[:, 0:2] @ f(inp["dD_w3"])).T

    # --- a9 (9, 73): W4e 0:7 | Bbot 7:63 | bc3C 63 | b4 [0,64] | ones [0,65:73]
    a9 = np.zeros((9, 73), np.float32)
    a9[0:8, 0:7] = f(inp["cC_w4"])
    a9[0:8, 7:63] = _build_bbot()
    a9[0:8, 63] = f(inp["cC_b3"])
    a9[0, 64] = f(inp["cC_b4"])[0]
    a9[0, 65:73] = 1.0

    return {
        "a14": np.ascontiguousarray(a14), "a128": np.ascontiguousarray(a128),
        "a33": a33, "a9": a9,
    }


# ---------------------------------------------------------------------------
# bass program
# ---------------------------------------------------------------------------
_NC_CACHE = {}


def _build_nc():
    import concourse.bass as bass
    import concourse.bacc as bacc
    import concourse.mybir as mybir
    import concourse.tile as tile_mod
    from concourse.tile import TileContext
    from concourse.vector_clock import ScopedClock

    # Slim kernel tail: keep the completion drain (waits for the out-DMA
    # queue, guaranteeing the output is in DRAM before the NEFF retires)
    # but skip the two all-engine barriers + per-sem zeroing. Safe here
    # because the Bass preamble dma_reset/sem_clears the full kernel sem
    # range at the START of every execution, so re-exec state is clean.
    def _slim_drain_and_barrier(self, tick_clock, wait_clock):
        drain_inst = self.nc.sync.drain()
        wait_clock.add_sem_waits(
            drain_inst.ins, ScopedClock({None: tick_clock.global_clock})
        )
        popped = self.nc._tile_sem_poison_stack.pop()
        assert popped is self._sem_poison

    F32 = mybir.dt.float32
    ADD = mybir.AluOpType.add
    MAX = mybir.AluOpType.max

    # Bacc (not raw Bass): its compile() runs move_matmul_waits_to_ldweights
    # + generate_event_semaphores, which split multi-wait instructions into
    # the 1-wait form TRN2 walrus codegen requires.
    nc = bacc.Bacc()
    d14 = nc.dram_tensor("a14", [14, 264], F32, kind="ExternalInput")
    d128 = nc.dram_tensor("a128", [128, 487], F32, kind="ExternalInput")
    d33 = nc.dram_tensor("a33", [33, 528], F32, kind="ExternalInput")
    d9 = nc.dram_tensor("a9", [9, 73], F32, kind="ExternalInput")
    dout = nc.dram_tensor("out", [1, 8], F32, kind="ExternalOutput")

    def ap(base, offset_delta, dims):
        """Custom AP on the same tensor as `base`, free dims replaced."""
        return bass.AP(tensor=base.tensor, offset=base.offset + offset_delta,
                       ap=[base.ap[0]] + dims)

    F32R_ = mybir.dt.float32r

    def r32(x):
        # producers feeding float32r matmuls must emit float32r
        return x.bitcast(F32R_)

    TileContext._drain_and_barrier = _slim_drain_and_barrier
    with TileContext(nc) as tc:
        with (
            tc.tile_pool(name="sb", bufs=1) as sb,
            tc.tile_pool(name="ps", bufs=8, space="PSUM") as ps,
        ):
            # ---- input DMAs, spread over the two HWDGE queues (SP + ACT);
            # each engine's queue runs serially, so order by need time ----
            # a14 gates the whole chain. A tiny sacrificial DMA ahead of it
            # absorbs the sync queue's first-completion semaphore penalty
            # (~2.0us vs ~1.4us warm), so a14's sem fires sooner. Nothing
            # reads qwarm.
            qwarm = sb.tile([1, 1], F32, tag="qwarm")
            nc.sync.dma_start(out=r32(qwarm), in_=r32(d9[0:1, 0:1]))
            a14 = sb.tile([14, 264], F32, tag="a14")
            nc.sync.dma_start(out=r32(a14), in_=r32(d14[:, :]))
            a128 = sb.tile([128, 487], F32, tag="a128")
            nc.scalar.dma_start(out=r32(a128), in_=r32(d128[:, :]))
            a33 = sb.tile([33, 528], F32, tag="a33")
            nc.scalar.dma_start(out=r32(a33), in_=r32(d33[:, :]))
            a9 = sb.tile([9, 73], F32, tag="a9")
            nc.scalar.dma_start(out=r32(a9), in_=r32(d9[:, :]))


            # warm ACT's activation table off the critical path (first
            # ACTIVATE pays a ~1.8us table load; do it on scratch now)
            warm = sb.tile([1, 1], F32, tag="warm")
            nc.scalar.mul(warm, warm, 0.0)

            F32R = mybir.dt.float32r

            def mm(out, lhsT, rhs, start, stop, r=True):
                # float32r: single-pass PE fp32 (vs 2 half-speed passes)
                if r:
                    lhsT = lhsT.bitcast(F32R)
                    rhs = rhs.bitcast(F32R)
                nc.tensor.matmul(out, lhsT=lhsT, rhs=rhs, start=start, stop=stop)

            ts = nc.vector.tensor_scalar
            cp = nc.vector.tensor_copy
            cpa = nc.scalar.copy  # prep copies go on ACT to keep DVE clear

            # ---- stage A: demand MLP 14 -> 256 -> 32 ----
            p1a = ps.tile([128, 8], F32, tag="ps")
            mm(p1a, lhsT=a14[0:14, 8:136], rhs=a14[0:14, 0:8], start=True, stop=True)
            p1b = ps.tile([128, 8], F32, tag="ps")
            mm(p1b, lhsT=a14[0:14, 136:264], rhs=a14[0:14, 0:8], start=True, stop=True)
            h1a = sb.tile([128, 8], F32, tag="h1a")
            ts(r32(h1a), p1a, a128[:, 224:225], 0.0, op0=ADD, op1=MAX)
            h1b = sb.tile([128, 8], F32, tag="h1b")
            nc.scalar.activation(r32(h1b), p1b, mybir.ActivationFunctionType.Relu, bias=a128[:, 225:226])

            p2 = ps.tile([32, 8], F32, tag="ps")
            mm(p2, lhsT=a128[0:128, 0:32], rhs=h1a, start=True, stop=False)
            mm(p2, lhsT=a128[0:128, 32:64], rhs=h1b, start=False, stop=True)

            # h2dup (33, 16): cols 0:8 = relu(h2)[:, piA], cols 8:16 = [:, piB],
            # row 32 = ones
            h2dup = sb.tile([33, 16], F32, tag="h2dup")
            ts(r32(h2dup[32:33, 0:16]), a14[0:1, 0:16], 0.0, 1.0,
               op0=mybir.AluOpType.mult, op1=ADD)
            # top gather [0,0,1,1,2,2,3,3]: in [[1,4],[0,2]], out [[2,4],[1,2]]
            ts(r32(ap(h2dup[0:32, 0:8], 0, [[2, 4], [1, 2]])),
               ap(p2[0:32, 0:8], 0, [[1, 4], [0, 2]]),
               a33[0:32, 4:5], 0.0, op0=ADD, op1=MAX)
            # bot gather [4,5,4,5,6,7,6,7]: in off4 [[2,2],[0,2],[1,2]],
            # out off8 [[4,2],[2,2],[1,2]]
            ts(r32(ap(h2dup[0:32, 0:8], 8, [[4, 2], [2, 2], [1, 2]])),
               ap(p2[0:32, 0:8], 4, [[2, 2], [0, 2], [1, 2]]),
               a33[0:32, 4:5], 0.0, op0=ADD, op1=MAX)

            # ---- fused demand-L3 + pair gather + coupled-L1 ----
            pB1 = ps.tile([128, 8], F32, tag="ps")
            mm(pB1, lhsT=a33[0:33, 16:144], rhs=h2dup[0:33, 0:8], start=True, stop=False)
            mm(pB1, lhsT=a33[0:33, 144:272], rhs=h2dup[0:33, 8:16], start=False, stop=True)
            hc1 = sb.tile([128, 8], F32, tag="hc1")
            ts(r32(hc1), pB1, a128[:, 228:229], 0.0, op0=ADD, op1=MAX)

            pB2 = ps.tile([32, 8], F32, tag="ps")
            mm(pB2, lhsT=a128[0:128, 192:224], rhs=hc1, start=True, stop=True)
            hc2e = sb.tile([33, 8], F32, tag="hc2e")
            ts(r32(hc2e[32:33, :]), a14[0:1, 0:8], 0.0, 1.0,
               op0=mybir.AluOpType.mult, op1=ADD)
            ts(r32(hc2e[0:32, :]), pB2, a33[0:32, 6:7], 0.0, op0=ADD, op1=MAX)

            # ---- dp / dpT and the D tensor (8, 56) ----
            pdp = ps.tile([8, 4], F32, tag="ps")
            mm(pdp, lhsT=hc2e[0:33, :], rhs=a33[0:33, 0:4], start=True, stop=True)
            pdpT = ps.tile([4, 8], F32, tag="ps")
            mm(pdpT, lhsT=a33[0:33, 0:4], rhs=hc2e[0:33, :], start=True, stop=True)

            dp_s = sb.tile([8, 4], F32, tag="dp_s")
            cp(out=r32(dp_s), in_=pdp)
            # D as (36, 56): top 4 channels at rows 0:4, bottom 4 at rows
            # 32:36 (legal DVE write base), rows 4:32 zeroed; the matching
            # WD1T36 weight rows are zero-padded on the host.
            D36 = sb.tile([36, 56], F32, tag="D36")
            ts(r32(D36[0:32, :]), a128[0:32, 0:56], 0.0, None,
               op0=mybir.AluOpType.mult)
            cp(out=r32(ap(D36[0:4, 0:56], 0, [[7, 8], [1, 7]])),
               in_=ap(pdpT[0:4, 0:8], 0, [[1, 8], [0, 7]]))
            pDb = ps.tile([4, 56], F32, tag="ps")
            mm(pDb, lhsT=dp_s, rhs=a9[0:8, 7:63], start=True, stop=True)
            cp(out=r32(D36[32:36, :]), in_=pDb)

            # ---- Conv_D: 8 -> 256 -> 32 ----
            pD1a = ps.tile([128, 56], F32, tag="ps")
            mm(pD1a, lhsT=a128[0:36, 231:359], rhs=D36, start=True, stop=True)
            pD1b = ps.tile([128, 56], F32, tag="ps")
            mm(pD1b, lhsT=a128[0:36, 359:487], rhs=D36, start=True, stop=True)
            hd1a = sb.tile([128, 56], F32, tag="hd1a")
            ts(r32(hd1a), pD1a, a128[:, 226:227], 0.0, op0=ADD, op1=MAX)
            hd1b = sb.tile([128, 56], F32, tag="hd1b")
            nc.scalar.activation(r32(hd1b), pD1b, mybir.ActivationFunctionType.Relu, bias=a128[:, 227:228])

            pD2 = ps.tile([32, 56], F32, tag="ps")
            mm(pD2, lhsT=a128[0:128, 64:96], rhs=hd1a, start=True, stop=False)
            mm(pD2, lhsT=a128[0:128, 96:128], rhs=hd1b, start=False, stop=True)
            hd2 = sb.tile([32, 56], F32, tag="hd2")
            ts(r32(hd2), pD2, a33[0:32, 5:6], 0.0, op0=ADD, op1=MAX)

            # ---- fused Conv_D-L3 + Conv_C-L1: 32 -> 256 ----
            pC1a = ps.tile([128, 56], F32, tag="ps")
            mm(pC1a, lhsT=a33[0:32, 272:400], rhs=hd2, start=True, stop=True)
            pC1b = ps.tile([128, 56], F32, tag="ps")
            mm(pC1b, lhsT=a33[0:32, 400:528], rhs=hd2, start=True, stop=True)
            hq1a = sb.tile([128, 56], F32, tag="hq1a")
            ts(r32(hq1a), pC1a, a128[:, 229:230], 0.0, op0=ADD, op1=MAX)
            hq1b = sb.tile([128, 56], F32, tag="hq1b")
            nc.scalar.activation(r32(hq1b), pC1b, mybir.ActivationFunctionType.Relu, bias=a128[:, 230:231])

            # ---- Conv_C: 256 -> 32 -> 8 ----
            pC2 = ps.tile([32, 56], F32, tag="ps")
            mm(pC2, lhsT=a128[0:128, 128:160], rhs=hq1a, start=True, stop=False)
            mm(pC2, lhsT=a128[0:128, 160:192], rhs=hq1b, start=False, stop=True)
            hq2 = sb.tile([32, 56], F32, tag="hq2")
            ts(r32(hq2), pC2, a33[0:32, 7:8], 0.0, op0=ADD, op1=MAX)

            pC3 = ps.tile([8, 56], F32, tag="ps")
            mm(pC3, lhsT=a33[0:32, 8:16], rhs=hq2, start=True, stop=True)
            hf = sb.tile([8, 56], F32, tag="hf")
            ts(r32(hf[0:8, :]), pC3, a9[0:8, 63:64], 0.0, op0=ADD, op1=MAX)

            # ---- final (1,7) conv: bias matmul (no data deps, issues early)
            # then 7 accumulating matmuls over the w offsets ----
            pout = ps.tile([1, 8], F32, tag="ps")
            mm(pout, lhsT=a9[0:1, 64:65], rhs=a9[0:1, 65:73],
               start=True, stop=False)
            for w in range(7):
                mm(pout, lhsT=a9[0:8, w:w + 1],
                   rhs=ap(hf[0:8, 0:8], w, [[7, 8]]),
                   start=False, stop=(w == 6))
            out_s = sb.tile([1, 8], F32, tag="out_s")
            cp(out=out_s, in_=pout)
            nc.sync.dma_start(out=dout[:, :], in_=out_s)

    nc.compile()
    return nc


def _get_nc():
    if "nc" not in _NC_CACHE:
        _NC_CACHE["nc"] = _build_nc()
    return _NC_CACHE["nc"]


# ---------------------------------------------------------------------------
# public entry points
# ---------------------------------------------------------------------------
def _unpermute(raw8):
    out = np.empty(8, np.float32)
    out[_ORIGROW] = raw8
    return out


def run_spmd(inputs, trace=False):
    """Run on the 8 NeuronCores; returns (output(8,), BassKernelResults)."""
    from concourse.bass_utils import run_bass_kernel_spmd
    packed = _host_pack(inputs)
    nc = _get_nc()
    core_ids = list(range(8))
    in_maps = [dict(packed) for _ in core_ids]
    res = run_bass_kernel_spmd(nc, in_maps, core_ids, trace=trace)
    out = _unpermute(np.asarray(res.results[0]["out"], np.float32)[0])
    return out, res


def run_sim(inputs):
    """CoreSim path (no hardware) for debugging."""
    from concourse import bass_interp
    packed = _host_pack(inputs)
    nc = _build_nc()
    sim = bass_interp.CoreSim(nc)
    for k, v in packed.items():
        sim.tensor(k)[:] = v
    sim.simulate()
    return _unpermute(np.asarray(sim.tensor("out"), np.float32)[0])


def kernel(**inputs):
    out, _ = run_spmd(inputs, trace=False)
    return out


# revision 42
# speedup vs baseline: 1.0553x; 1.0253x over previous
"""Trainium2 Bass kernel for nn_FRAP_47047071761018.

FRAP forward pass (demand MLP -> coupled-demand MLP -> D-tensor -> two
1x1-conv stacks -> (1,7) conv). Batch=1, tiny dims => latency problem.
Strategy: replicate the whole computation on all 8 cores (no useful
tensor sharding), read the result from core 0.

Device dataflow (float32r, features-on-partitions orientation):
  - every Linear is out = lhsT.T @ rhs on the PE with K on partitions;
    operands are float32r (single-pass PE fp32, ~2x the fp32 rate)
  - relu(x+b) fused into one DVE tensor_scalar (op0=add per-partition
    bias, op1=max 0); the second relu of 128-row pairs runs on ACT
  - host folds (weight preprocessing): demand-L3 + pair-gather +
    coupled-L1 -> FaT/FbT = (c_w1[:, :4] @ [d_w3|d_b3])^T, and
    Conv_D-L3 + Conv_C-L1 -> WeffT = (cC_w1[:, :2] @ dD_w3)^T with
    beff = cC_w1[:, :2] @ dD_b3 + cC_b1
  - D tensor as a (36, 56) tile: top channels at rows 0:4, bottom at
    rows 32:36 (legal DVE write bases), zero rows between, matched by
    zero-padded WD1T36 weights -> Conv_D L1 is 2 matmuls
  - final (1,7) conv = 7 accumulating matmuls with stride-7 rhs APs,
    bias via an extra ones-partition row
  - inputs arrive as 4 packed DRAM arrays over both HWDGE queues
    (SP + ACT), ordered by need time; the Tile exit tail is slimmed to
    just the completion drain (the Bass preamble re-clears semaphores
    at the start of every execution)
Pair rows are processed in a permuted order (PI) chosen so both lane
gathers are strided APs; the host unpermutes the 8 outputs at the end.
"""

import numpy as np

# ---------------------------------------------------------------------------
# static structure (from the FRAP definition; hardcoded, no file reads)
# ---------------------------------------------------------------------------
# reference PAIRS rows:
_PAIRS = [(0, 4), (1, 5), (2, 6), (3, 7), (0, 5), (1, 4), (2, 7), (3, 6)]
# our processing order: both column projections are strided-AP sequences
_PI_PAIRS = [(0, 4), (0, 5), (1, 4), (1, 5), (2, 6), (2, 7), (3, 6), (3, 7)]
_ORIGROW = [_PAIRS.index(p) for p in _PI_PAIRS]          # p' -> original row
_POS_OF_ORIG = np.argsort(_ORIGROW)                      # original row -> p'


def _build_bbot():
    """Bbot[q, p'*7+k] = 1 iff ORIGROW[q] == j(ORIGROW[p'], k), j=k+(k>=i)."""
    B = np.zeros((8, 56), np.float32)
    for pp in range(8):
        i = _ORIGROW[pp]
        for k in range(7):
            jj = k + (1 if k >= i else 0)
            B[_POS_OF_ORIG[jj], pp * 7 + k] = 1.0
    return B


def _host_pack(inp):
    """Marshal kernel inputs into the 7 packed DRAM arrays the device
    program expects (transposes / concats / 0-1 selection constants)."""
    f = lambda x: np.ascontiguousarray(np.asarray(x, np.float32))
    # --- a14 (14, 264): XT | W1dT ---
    XT = np.concatenate([
        np.broadcast_to(f(inp["i_position"])[0, 0], (1, 8)),
        np.broadcast_to(f(inp["j_position"])[0, 0], (1, 8)),
        f(inp["wtime_mu"]), f(inp["wtime_sigma"]),
        f(inp["wtime_max"]), f(inp["waiting"]),
        np.broadcast_to(f(inp["phase"]).T, (8, 8)),
    ], axis=0)
    # padded to 128 partitions: a 14-partition DMA only engages 14/128 of
    # the SBUF write ports (~1.0us); the 128-row form streams at full port
    # parallelism (~0.7us) and the tile is 128 partitions anyway
    a14 = np.zeros((128, 264), np.float32)
    a14[0:14] = np.concatenate([XT, f(inp["d_w1"]).T], axis=1)

    # --- a128 (128, 229): W2dT(64) WD2T(64) WC2T(64) Wc2T(32) b1(2) bD1(2) bc1(1)
    w2t = f(inp["d_w2"]).T
    wd2t = f(inp["dD_w2"]).T
    wc2t = f(inp["cC_w2"]).T
    beff = (f(inp["cC_w1"])[:, 0:2] @ f(inp["dD_b3"]) + f(inp["cC_b1"]))
    a128 = np.concatenate([
        w2t[:128], w2t[128:], wd2t[:128], wd2t[128:], wc2t[:128], wc2t[128:],
        f(inp["c_w2"]).T,
        f(inp["d_b1"]).reshape(2, 128).T, f(inp["dD_b1"]).reshape(2, 128).T,
        f(inp["c_b1"]).reshape(128, 1),
        beff.reshape(2, 128).T,
    ], axis=1)                                                    # (128, 231)
    wd1t = f(inp["dD_w1"]).T                                      # (8, 256)
    wd1t36 = np.zeros((128, 256), np.float32)
    wd1t36[0:4] = wd1t[0:4]
    wd1t36[32:36] = wd1t[4:8]
    a128 = np.concatenate([a128, wd1t36], axis=1)                 # (128, 487)

    # --- a33 (33, 528): [Wc3Te(4) b2 bD2 bc2 bc2C WcC3T(8)] | FaT(128) | FbT(128)
    #     | WeffT(256)  (host-folded weight products)
    a33 = np.zeros((33, 528), np.float32)
    a33[0:32, 0:4] = f(inp["c_w3"]).T[0:32]
    a33[32, 0:4] = f(inp["c_b3"])
    a33[0:32, 4] = f(inp["d_b2"])
    a33[0:32, 5] = f(inp["dD_b2"])
    a33[0:32, 6] = f(inp["c_b2"])
    a33[0:32, 7] = f(inp["cC_b2"])
    a33[0:32, 8:16] = f(inp["cC_w3"]).T
    w3aug = np.concatenate([f(inp["d_w3"]), f(inp["d_b3"])[:, None]], axis=1)  # (4,33)
    wc1 = f(inp["c_w1"])                                          # (128, 8)
    a33[0:33, 16:144] = (wc1[:, 0:4] @ w3aug).T
    a33[0:33, 144:272] = (wc1[:, 4:8] @ w3aug).T
    a33[0:32, 272:528] = (f(inp["cC_w1"])[:, 0:2] @ f(inp["dD_w3"])).T

    # --- a9 (9, 73): W4e 0:7 | Bbot 7:63 | bc3C 63 | b4 [0,64] | ones [0,65:73]
    a9 = np.zeros((9, 73), np.float32)
    a9[0:8, 0:7] = f(inp["cC_w4"])
    a9[0:8, 7:63] = _build_bbot()
    a9[0:8, 63] = f(inp["cC_b3"])
    a9[0, 64] = f(inp["cC_b4"])[0]
    a9[0, 65:73] = 1.0

    return {
        "a14": np.ascontiguousarray(a14), "a128": np.ascontiguousarray(a128),
        "a33": a33, "a9": a9,
    }


# ---------------------------------------------------------------------------
# bass program
# ---------------------------------------------------------------------------
_NC_CACHE = {}


def _build_nc():
    import concourse.bass as bass
    import concourse.bacc as bacc
    import concourse.mybir as mybir
    import concourse.tile as tile_mod
    from concourse.tile import TileContext
    from concourse.vector_clock import ScopedClock

    # Slim kernel tail: keep the completion drain (waits for the out-DMA
    # queue, guaranteeing the output is in DRAM before the NEFF retires)
    # but skip the two all-engine barriers + per-sem zeroing. Safe here
    # because the Bass preamble dma_reset/sem_clears the full kernel sem
    # range at the START of every execution, so re-exec state is clean.
    def _slim_drain_and_barrier(self, tick_clock, wait_clock):
        drain_inst = self.nc.sync.drain()
        wait_clock.add_sem_waits(
            drain_inst.ins, ScopedClock({None: tick_clock.global_clock})
        )
        popped = self.nc._tile_sem_poison_stack.pop()
        assert popped is self._sem_poison

    F32 = mybir.dt.float32
    ADD = mybir.AluOpType.add
    MAX = mybir.AluOpType.max

    # Bacc (not raw Bass): its compile() runs move_matmul_waits_to_ldweights
    # + generate_event_semaphores, which split multi-wait instructions into
    # the 1-wait form TRN2 walrus codegen requires.
    nc = bacc.Bacc()
    d14 = nc.dram_tensor("a14", [128, 264], F32, kind="ExternalInput")
    d128 = nc.dram_tensor("a128", [128, 487], F32, kind="ExternalInput")
    d33 = nc.dram_tensor("a33", [33, 528], F32, kind="ExternalInput")
    d9 = nc.dram_tensor("a9", [9, 73], F32, kind="ExternalInput")
    dout = nc.dram_tensor("out", [1, 8], F32, kind="ExternalOutput")

    def ap(base, offset_delta, dims):
        """Custom AP on the same tensor as `base`, free dims replaced."""
        return bass.AP(tensor=base.tensor, offset=base.offset + offset_delta,
                       ap=[base.ap[0]] + dims)

    F32R_ = mybir.dt.float32r

    def r32(x):
        # producers feeding float32r matmuls must emit float32r
        return x.bitcast(F32R_)

    TileContext._drain_and_barrier = _slim_drain_and_barrier
    with TileContext(nc) as tc:
        with (
            tc.tile_pool(name="sb", bufs=1) as sb,
            tc.tile_pool(name="ps", bufs=8, space="PSUM") as ps,
        ):
            # ---- input DMAs, spread over the two HWDGE queues (SP + ACT);
            # each engine's queue runs serially, so order by need time ----
            # a14 gates the whole chain; keep it ALONE on the sync queue —
            # completion sems only become visible once a queue drains, so a
            # second sync-queue DMA would push the chain start out with it.
            a14 = sb.tile([128, 264], F32, tag="a14")
            nc.sync.dma_start(out=r32(a14), in_=r32(d14[:, :]))
            a128 = sb.tile([128, 487], F32, tag="a128")
            nc.scalar.dma_start(out=r32(a128), in_=r32(d128[:, :]))
            a33 = sb.tile([33, 528], F32, tag="a33")
            nc.scalar.dma_start(out=r32(a33), in_=r32(d33[:, :]))
            a9 = sb.tile([9, 73], F32, tag="a9")
            nc.scalar.dma_start(out=r32(a9), in_=r32(d9[:, :]))


            # warm ACT's activation table off the critical path (first
            # ACTIVATE pays a ~1.8us table load; do it on scratch now)
            warm = sb.tile([1, 1], F32, tag="warm")
            nc.scalar.mul(warm, warm, 0.0)

            F32R = mybir.dt.float32r

            def mm(out, lhsT, rhs, start, stop, r=True):
                # float32r: single-pass PE fp32 (vs 2 half-speed passes)
                if r:
                    lhsT = lhsT.bitcast(F32R)
                    rhs = rhs.bitcast(F32R)
                nc.tensor.matmul(out, lhsT=lhsT, rhs=rhs, start=start, stop=stop)

            ts = nc.vector.tensor_scalar
            cp = nc.vector.tensor_copy
            cpa = nc.scalar.copy  # prep copies go on ACT to keep DVE clear

            # ---- stage A: demand MLP 14 -> 256 -> 32 ----
            p1a = ps.tile([128, 8], F32, tag="ps")
            mm(p1a, lhsT=a14[0:14, 8:136], rhs=a14[0:14, 0:8], start=True, stop=True)
            p1b = ps.tile([128, 8], F32, tag="ps")
            mm(p1b, lhsT=a14[0:14, 136:264], rhs=a14[0:14, 0:8], start=True, stop=True)
            h1a = sb.tile([128, 8], F32, tag="h1a")
            ts(r32(h1a), p1a, a128[:, 224:225], 0.0, op0=ADD, op1=MAX)
            h1b = sb.tile([128, 8], F32, tag="h1b")
            nc.scalar.activation(r32(h1b), p1b, mybir.ActivationFunctionType.Relu, bias=a128[:, 225:226])

            p2 = ps.tile([32, 8], F32, tag="ps")
            mm(p2, lhsT=a128[0:128, 0:32], rhs=h1a, start=True, stop=False)
            mm(p2, lhsT=a128[0:128, 32:64], rhs=h1b, start=False, stop=True)

            # h2dup (33, 16): cols 0:8 = relu(h2)[:, piA], cols 8:16 = [:, piB],
            # row 32 = ones
            h2dup = sb.tile([33, 16], F32, tag="h2dup")
            ts(r32(h2dup[32:33, 0:16]), a14[0:1, 0:16], 0.0, 1.0,
               op0=mybir.AluOpType.mult, op1=ADD)
            # top gather [0,0,1,1,2,2,3,3]: in [[1,4],[0,2]], out [[2,4],[1,2]]
            ts(r32(ap(h2dup[0:32, 0:8], 0, [[2, 4], [1, 2]])),
               ap(p2[0:32, 0:8], 0, [[1, 4], [0, 2]]),
               a33[0:32, 4:5], 0.0, op0=ADD, op1=MAX)
            # bot gather [4,5,4,5,6,7,6,7]: in off4 [[2,2],[0,2],[1,2]],
            # out off8 [[4,2],[2,2],[1,2]]
            ts(r32(ap(h2dup[0:32, 0:8], 8, [[4, 2], [2, 2], [1, 2]])),
               ap(p2[0:32, 0:8], 4, [[2, 2], [0, 2], [1, 2]]),
               a33[0:32, 4:5], 0.0, op0=ADD, op1=MAX)

            # ---- fused demand-L3 + pair gather + coupled-L1 ----
            pB1 = ps.tile([128, 8], F32, tag="ps")
            mm(pB1, lhsT=a33[0:33, 16:144], rhs=h2dup[0:33, 0:8], start=True, stop=False)
            mm(pB1, lhsT=a33[0:33, 144:272], rhs=h2dup[0:33, 8:16], start=False, stop=True)
            hc1 = sb.tile([128, 8], F32, tag="hc1")
            ts(r32(hc1), pB1, a128[:, 228:229], 0.0, op0=ADD, op1=MAX)

            pB2 = ps.tile([32, 8], F32, tag="ps")
            mm(pB2, lhsT=a128[0:128, 192:224], rhs=hc1, start=True, stop=True)
            hc2e = sb.tile([33, 8], F32, tag="hc2e")
            ts(r32(hc2e[32:33, :]), a14[0:1, 0:8], 0.0, 1.0,
               op0=mybir.AluOpType.mult, op1=ADD)
            ts(r32(hc2e[0:32, :]), pB2, a33[0:32, 6:7], 0.0, op0=ADD, op1=MAX)

            # ---- dp / dpT and the D tensor (8, 56) ----
            pdp = ps.tile([8, 4], F32, tag="ps")
            mm(pdp, lhsT=hc2e[0:33, :], rhs=a33[0:33, 0:4], start=True, stop=True)
            pdpT = ps.tile([4, 8], F32, tag="ps")
            mm(pdpT, lhsT=a33[0:33, 0:4], rhs=hc2e[0:33, :], start=True, stop=True)

            dp_s = sb.tile([8, 4], F32, tag="dp_s")
            cp(out=r32(dp_s), in_=pdp)
            # D as (36, 56): top 4 channels at rows 0:4, bottom 4 at rows
            # 32:36 (legal DVE write base), rows 4:32 zeroed; the matching
            # WD1T36 weight rows are zero-padded on the host.
            D36 = sb.tile([36, 56], F32, tag="D36")
            ts(r32(D36[0:32, :]), a128[0:32, 0:56], 0.0, None,
               op0=mybir.AluOpType.mult)
            cp(out=r32(ap(D36[0:4, 0:56], 0, [[7, 8], [1, 7]])),
               in_=ap(pdpT[0:4, 0:8], 0, [[1, 8], [0, 7]]))
            pDb = ps.tile([4, 56], F32, tag="ps")
            mm(pDb, lhsT=dp_s, rhs=a9[0:8, 7:63], start=True, stop=True)
            cp(out=r32(D36[32:36, :]), in_=pDb)

            # ---- Conv_D: 8 -> 256 -> 32 ----
            pD1a = ps.tile([128, 56], F32, tag="ps")
            mm(pD1a, lhsT=a128[0:36, 231:359], rhs=D36, start=True, stop=True)
            pD1b = ps.tile([128, 56], F32, tag="ps")
            mm(pD1b, lhsT=a128[0:36, 359:487], rhs=D36, start=True, stop=True)
            hd1a = sb.tile([128, 56], F32, tag="hd1a")
            ts(r32(hd1a), pD1a, a128[:, 226:227], 0.0, op0=ADD, op1=MAX)
            hd1b = sb.tile([128, 56], F32, tag="hd1b")
            nc.scalar.activation(r32(hd1b), pD1b, mybir.ActivationFunctionType.Relu, bias=a128[:, 227:228])

            pD2 = ps.tile([32, 56], F32, tag="ps")
            mm(pD2, lhsT=a128[0:128, 64:96], rhs=hd1a, start=True, stop=False)
            mm(pD2, lhsT=a128[0:128, 96:128], rhs=hd1b, start=False, stop=True)
            hd2 = sb.tile([32, 56], F32, tag="hd2")
            ts(r32(hd2), pD2, a33[0:32, 5:6], 0.0, op0=ADD, op1=MAX)

            # ---- fused Conv_D-L3 + Conv_C-L1: 32 -> 256 ----
            pC1a = ps.tile([128, 56], F32, tag="ps")
            mm(pC1a, lhsT=a33[0:32, 272:400], rhs=hd2, start=True, stop=True)
            pC1b = ps.tile([128, 56], F32, tag="ps")
            mm(pC1b, lhsT=a33[0:32, 400:528], rhs=hd2, start=True, stop=True)
            hq1a = sb.tile([128, 56], F32, tag="hq1a")
            ts(r32(hq1a), pC1a, a128[:, 229:230], 0.0, op0=ADD, op1=MAX)
            hq1b = sb.tile([128, 56], F32, tag="hq1b")
            nc.scalar.activation(r32(hq1b), pC1b, mybir.ActivationFunctionType.Relu, bias=a128[:, 230:231])

            # ---- Conv_C: 256 -> 32 -> 8 ----
            pC2 = ps.tile([32, 56], F32, tag="ps")
            mm(pC2, lhsT=a128[0:128, 128:160], rhs=hq1a, start=True, stop=False)
            mm(pC2, lhsT=a128[0:128, 160:192], rhs=hq1b, start=False, stop=True)
            hq2 = sb.tile([32, 56], F32, tag="hq2")
            ts(r32(hq2), pC2, a33[0:32, 7:8], 0.0, op0=ADD, op1=MAX)

            pC3 = ps.tile([8, 56], F32, tag="ps")
            mm(pC3, lhsT=a33[0:32, 8:16], rhs=hq2, start=True, stop=True)
            hf = sb.tile([8, 56], F32, tag="hf")
            ts(r32(hf[0:8, :]), pC3, a9[0:8, 63:64], 0.0, op0=ADD, op1=MAX)

            # ---- final (1,7) conv: bias matmul (no data deps, issues early)
            # then 7 accumulating matmuls over the w offsets ----
            pout = ps.tile([1, 8], F32, tag="ps")
            mm(pout, lhsT=a9[0:1, 64:65], rhs=a9[0:1, 65:73],
               start=True, stop=False)
            for w in range(7):
                mm(pout, lhsT=a9[0:8, w:w + 1],
                   rhs=ap(hf[0:8, 0:8], w, [[7, 8]]),
                   start=False, stop=(w == 6))
            out_s = sb.tile([1, 8], F32, tag="out_s")
            cp(out=out_s, in_=pout)
            nc.sync.dma_start(out=dout[:, :], in_=out_s)

    nc.compile()
    return nc


def _get_nc():
    if "nc" not in _NC_CACHE:
        _NC_CACHE["nc"] = _build_nc()
    return _NC_CACHE["nc"]


# ---------------------------------------------------------------------------
# public entry points
# ---------------------------------------------------------------------------
def _unpermute(raw8):
    out = np.empty(8, np.float32)
    out[_ORIGROW] = raw8
    return out


def run_spmd(inputs, trace=False):
    """Run on the 8 NeuronCores; returns (output(8,), BassKernelResults)."""
    from concourse.bass_utils import run_bass_kernel_spmd
    packed = _host_pack(inputs)
    nc = _get_nc()
    core_ids = list(range(8))
    in_maps = [dict(packed) for _ in core_ids]
    res = run_bass_kernel_spmd(nc, in_maps, core_ids, trace=trace)
    out = _unpermute(np.asarray(res.results[0]["out"], np.float32)[0])
    return out, res


def run_sim(inputs):
    """CoreSim path (no hardware) for debugging."""
    from concourse import bass_interp
    packed = _host_pack(inputs)
    nc = _build_nc()
    sim = bass_interp.CoreSim(nc)
    for k, v in packed.items():
        sim.tensor(k)[:] = v
    sim.simulate()
    return _unpermute(np.asarray(sim.tensor("out"), np.float32)[0])


def kernel(**inputs):
    out, _ = run_spmd(inputs, trace=False)
    return out
